# revision 10
# baseline (speedup 1.0000x reference)
"""DiffOfGaussians Trainium2 kernel (v5: f16 stream, 2x-mode add trees).

Math:
  out[b,u] = sum_{h,w,c} inputs[b,h,w,c] * F[h,w,u] + bias[u]
  F[h,w,u] = g(a1,s1) - g(a2,s1+s2),  g(a,s) = a*exp(-((w-ux)^2+(h-uy)^2)/(2s))/(2*pi*s)

Separable filter: F[h,w,u] = Gx1[w,u]*gy1[u,h] + Gx2[w,u]*gy2[u,h] with the
amplitudes folded into gy and the minus sign into gy2.

Sharding: H split across 8 cores (16 rows each). The host pre-transposes each
slab to [h, w, b*c] float16: the DMA lands with w on partitions (half the HBM
bytes of f32) and the channel reduce is a stride-1 innermost halving tree of
tensor_tensor adds — measured to run in the DVE 2x f16 mode, unlike
tensor_reduce which stays at 1x on this hardware.

Per 4-row group: c-reduce tree (group 0 on GpSimd, rest on DVE) into
xt[w, h*64+b]; 4 f16 PE matmuls (2 paths x 2 unit-halves, paired per k into
one PSUM bank) produce pmm[u, (h,b)]; one wide DVE multiply per (k, group)
scales by gy[u,(p,h)] and scatters into slots[u, (b, hg,p,hh)]. A final
halving tree per k sums the 32 slot columns; bias/8 rides the f32 eviction
ACT. Host sums the 8 partial (2,128,64) outputs.
"""

import sys

for _p in ("/opt/trn_rl_repo",):
    if _p not in sys.path:
        sys.path.insert(0, _p)

import numpy as np

import concourse.bass as bass
import concourse.tile as tile
from concourse import bacc, masks, mybir
from concourse.bass_utils import run_bass_kernel_spmd

F32 = mybir.dt.float32
F16 = mybir.dt.float16
AX = mybir.AxisListType
OP = mybir.AluOpType
AF = mybir.ActivationFunctionType

B, H, W, C, U = 64, 128, 128, 16, 256
NCORES = 8
HSH = H // NCORES  # 16 rows per core
INV2PI = float(1.0 / (2.0 * np.pi))

_CACHE = {}


def _build_kernel():
    nc = bacc.Bacc(
        "TRN2",
        target_bir_lowering=False,
        debug=False,
        num_devices=NCORES,
    )

    # x transposed on host: x_d[h, w, b*c] (f16)
    x_d = nc.dram_tensor("x", [HSH, W, B * C], F16, kind="ExternalInput").ap()
    yc_d = nc.dram_tensor("yc", [1, HSH], F32, kind="ExternalInput").ap()
    # packed params: col 2i+k = param i, units k*128..k*128+127
    # order: a1, a2, s1, s2, ux, uy, bias (cols 12:14), pad to 16
    prm_d = nc.dram_tensor("prm", [128, 16], F32, kind="ExternalInput").ap()
    # out[k, u_lo, b] = partial of out[b, k*128+u_lo]
    out_d = nc.dram_tensor("out", [2, 128, 64], F32, kind="ExternalOutput").ap()

    GRP = 4  # h rows per group
    NG = HSH // GRP

    with tile.TileContext(nc) as tc:
        with (
            tc.tile_pool(name="singles", bufs=1) as singles,
            tc.tile_pool(name="gx", bufs=4) as gx_pool,
            tc.tile_pool(name="tree", bufs=2) as tree_pool,
            tc.tile_pool(name="ptr", bufs=2, space="PSUM") as tr_psum,
            tc.tile_pool(name="pmm", bufs=2, space="PSUM") as mm_psum,
        ):
            # input stream: 4 batched issues (4 h-rows each); the sync engine
            # takes ~660ns per DMA_DIRECT2D so 16 issues would stagger the
            # stream start over ~10us.
            xin = singles.tile([128, HSH * B * C], F16)
            hrow = W * B * C  # elements per h row in x_d
            for g in range(NG):
                src = bass.AP(
                    tensor=x_d.tensor,
                    offset=x_d.offset + g * GRP * hrow,
                    ap=[[B * C, W], [hrow, GRP], [1, B * C]],
                )
                nc.sync.dma_start(
                    out=xin[:, g * GRP * B * C : (g + 1) * GRP * B * C], in_=src
                )

            # ---------------- constants & parameters ----------------
            identity = singles.tile([128, 128], F32)
            masks.make_identity(nc, identity[:])

            zbias = singles.tile([128, 1], F32)
            nc.vector.memset(zbias[:], 0.0)

            iota_i = singles.tile([128, 128], mybir.dt.int32)
            nc.gpsimd.iota(iota_i[:], pattern=[[1, 128]], base=0, channel_multiplier=0)
            iota_f = singles.tile([128, 128], F32)
            nc.vector.tensor_copy(iota_f[:], iota_i[:])

            prm_sb = singles.tile([128, 16], F32)
            nc.scalar.dma_start(out=prm_sb[:], in_=prm_d)
            _ord = ("a1", "a2", "s1", "s2", "ux", "uy")
            psb = {n: prm_sb[:, 2 * i : 2 * i + 2] for i, n in enumerate(_ord)}
            bias_sb = prm_sb[:, 12:14]

            yc_sb = singles.tile([128, HSH], F32)
            yc_bcast = bass.AP(
                tensor=yc_d.tensor, offset=yc_d.offset, ap=[[0, 128], [1, HSH]]
            )
            nc.gpsimd.dma_start(out=yc_sb[:], in_=yc_bcast)

            # derived per-unit params, all [128, 2]
            sig2 = singles.tile([128, 2], F32)
            nc.vector.tensor_add(sig2[:], psb["s1"], psb["s2"])
            rc1 = singles.tile([128, 2], F32)
            nc.vector.reciprocal(rc1[:], psb["s1"])
            rc2 = singles.tile([128, 2], F32)
            nc.vector.reciprocal(rc2[:], sig2[:])
            nis = []  # -1/(2 sigma_path)
            for p, rc in enumerate((rc1, rc2)):
                t = singles.tile([128, 2], F32, tag=f"nis{p}")
                nc.vector.tensor_scalar_mul(t[:], rc[:], -0.5)
                nis.append(t)
            # amplitude coefs: c1 = a1/(2 pi s1), c2n = -a2/(2 pi (s1+s2))
            coef = []
            for p, (a, rc, s) in enumerate(
                ((psb["a1"], rc1, INV2PI), (psb["a2"], rc2, -INV2PI))
            ):
                t0 = singles.tile([128, 2], F32, tag=f"coefa{p}")
                nc.vector.tensor_mul(t0[:], a, rc[:])
                t1 = singles.tile([128, 2], F32, tag=f"coef{p}")
                nc.vector.tensor_scalar_mul(t1[:], t0[:], s)
                coef.append(t1)

            # ---------------- Gx tables: gxw[p][w, u] (f16) ----------------
            nux = singles.tile([128, 2], F32)
            nc.vector.tensor_scalar_mul(nux[:], psb["ux"], -1.0)
            nuy = singles.tile([128, 2], F32)
            nc.vector.tensor_scalar_mul(nuy[:], psb["uy"], -1.0)
            dx2 = []
            for k in range(2):
                d2 = singles.tile([128, 128], F32, tag=f"dx2_{k}")
                nc.scalar.activation(
                    d2[:], iota_f[:], AF.Square, bias=nux[:, k : k + 1]
                )
                dx2.append(d2)

            gxw = []  # per path: [128(w), 256(u)] f16
            for p in range(2):
                t = singles.tile([128, 256], F16, tag=f"gxw{p}")
                gxw.append(t)
            for p in range(2):
                for k in range(2):
                    g = gx_pool.tile([128, 128], F32, tag="gx")
                    nc.scalar.activation(
                        g[:], dx2[k][:], AF.Exp,
                        bias=zbias[:, 0:1], scale=nis[p][:, k : k + 1],
                    )
                    ps = tr_psum.tile([128, 128], F32)
                    nc.tensor.transpose(ps[:], g[:], identity[:])
                    nc.scalar.copy(gxw[p][:, k * 128 : (k + 1) * 128], ps[:])

            # ----- gy table: gy_all[u_lo, k*32 + p*16 + h] (f32) -----
            gy_all = singles.tile([128, 64], F32)
            for k in range(2):
                dy2 = gx_pool.tile([128, HSH], F32, tag="dy2")
                nc.scalar.activation(
                    dy2[:], yc_sb[:], AF.Square, bias=nuy[:, k : k + 1]
                )
                for p in range(2):
                    e = gx_pool.tile([128, HSH], F32, tag="gye")
                    nc.scalar.activation(
                        e[:], dy2[:], AF.Exp,
                        bias=zbias[:, 0:1], scale=nis[p][:, k : k + 1],
                    )
                    off = k * 32 + p * 16
                    nc.vector.tensor_scalar_mul(
                        gy_all[:, off : off + 16], e[:], coef[p][:, k : k + 1]
                    )

            bias8 = singles.tile([128, 2], F32)
            nc.vector.tensor_scalar_mul(bias8[:], bias_sb, 1.0 / NCORES)

            # xt[w, h*64+b] = sum_c x[h, w, b, c] (f16, matmul moving tensor)
            xt_all = singles.tile([128, HSH * 64], F16)
            # slots[k][u_lo, b*32 + hg*8 + p*4 + hh]
            slots = []
            for k in range(2):
                slot_k = singles.tile([128, 2048], F16, tag=f"slots{k}")
                slots.append(slot_k)

            def cred_tree(g, eng):
                """c-reduce rows [4g, 4g+4) via halving adds on `eng`."""
                n = GRP * B  # 256 (h,b) columns in this group
                src = xin[:, g * GRP * B * C : (g + 1) * GRP * B * C]
                cur = src.rearrange("q (n c) -> q n c", c=C)
                width = C
                while width > 2:
                    width //= 2
                    t = tree_pool.tile([128, n * width], F16, tag=f"tr{width}")
                    tv = t[:].rearrange("q (n c) -> q n c", c=width)
                    eng.tensor_add(tv, cur[:, :, 0:width], cur[:, :, width:])
                    cur = tv
                dst = xt_all[:, g * n : (g + 1) * n].rearrange(
                    "q (n c) -> q n c", c=1
                )
                eng.tensor_add(dst, cur[:, :, 0:1], cur[:, :, 1:2])

            def mm_group(hg):
                for k in range(2):
                    pmm = mm_psum.tile([128, 512], F32, tag="pmm")
                    for p in range(2):
                        nc.tensor.matmul(
                            pmm[:, p * 256 : (p + 1) * 256],
                            gxw[p][:, k * 128 : (k + 1) * 128],
                            xt_all[:, hg * 256 : (hg + 1) * 256],
                            start=True,
                            stop=True,
                        )
                    # wide multiplies: slots <- pmm * gy, dims (b, hh)
                    pq = pmm[:]
                    for p in range(2):
                        pv = bass.AP(
                            tensor=pq.tensor, offset=pq.offset + p * 256,
                            ap=[pq.ap[0], [1, 64], [64, 4]],
                        )
                        goff = k * 32 + p * 16 + hg * 4
                        gq = gy_all[:, goff : goff + 4]
                        gv = bass.AP(
                            tensor=gq.tensor, offset=gq.offset,
                            ap=[gq.ap[0], [0, 64], [1, 4]],
                        )
                        soff = hg * 8 + p * 4
                        sq = slots[k][:, soff : soff + 4]
                        sv = bass.AP(
                            tensor=sq.tensor, offset=sq.offset,
                            ap=[sq.ap[0], [32, 64], [1, 4]],
                        )
                        nc.vector.tensor_tensor(sv, pv, gv, op=OP.mult)

            with nc.allow_low_precision("f16 partials; harness gate is 2e-2"):
                for g in range(NG):
                    cred_tree(g, nc.vector)
                    mm_group(g)

                # ---------------- final reduce + bias + store ----------------
                out_sb = singles.tile([128, 128], F32)
                for k in range(2):
                    cur = slots[k][:].rearrange("q (n r) -> q n r", r=32)
                    width = 32
                    while width > 2:
                        width //= 2
                        t = tree_pool.tile([128, 64 * width], F16, tag=f"fin{width}")
                        tv = t[:].rearrange("q (n r) -> q n r", r=width)
                        nc.vector.tensor_add(
                            tv, cur[:, :, 0:width], cur[:, :, width:]
                        )
                        cur = tv
                    sumk = singles.tile([128, 64], F16, tag=f"sum{k}")
                    nc.vector.tensor_add(
                        sumk[:].rearrange("q (n r) -> q n r", r=1),
                        cur[:, :, 0:1],
                        cur[:, :, 1:2],
                    )
                    nc.scalar.activation(
                        out_sb[:, k * 64 : (k + 1) * 64], sumk[:],
                        AF.Identity, bias=bias8[:, k : k + 1],
                    )
                    nc.sync.dma_start(
                        out=out_d[k], in_=out_sb[:, k * 64 : (k + 1) * 64]
                    )

    nc.compile()
    return nc


def _get_nc():
    if "nc" not in _CACHE:
        _CACHE["nc"] = _build_kernel()
    return _CACHE["nc"]


def pack_params(inputs: dict) -> np.ndarray:
    """[128, 16]: col 2i+k = param i (a1,a2,s1,s2,ux,uy,bias), unit block k."""
    prm = np.zeros((128, 16), dtype=np.float32)
    names = ("a1", "a2", "s1", "s2", "ux", "uy", "bias")
    for i, n in enumerate(names):
        v = np.asarray(inputs[n], dtype=np.float32).reshape(U)
        prm[:, 2 * i] = v[:128]
        prm[:, 2 * i + 1] = v[128:]
    return prm


def run(inputs: dict, trace: bool = False):
    """Run on 8 cores; returns (full_output, BassKernelResults)."""
    nc = _get_nc()
    x = np.asarray(inputs["inputs"], dtype=np.float32)
    # [b, h, w, c] -> [h, w, b, c], cast f16; per-core slabs are contiguous
    xt = np.ascontiguousarray(
        x.transpose(1, 2, 0, 3).astype(np.float16)
    ).reshape(H, W, B * C)
    prm = pack_params(inputs)
    in_maps = []
    for i in range(NCORES):
        m = {
            "x": xt[i * HSH : (i + 1) * HSH],
            "yc": np.arange(i * HSH, (i + 1) * HSH, dtype=np.float32).reshape(
                1, HSH
            ),
            "prm": prm,
        }
        in_maps.append(m)

    res = run_bass_kernel_spmd(
        nc, in_maps, core_ids=list(range(NCORES)), trace=trace
    )
    # partials: [2, 128, 64] -> out[b, k*128+u_lo]
    total = np.zeros((2, 128, 64), dtype=np.float64)
    for r in res.results:
        total += r["out"].astype(np.float64)
    out = total.transpose(2, 0, 1).reshape(64, 256).astype(np.float32)
    return out, res


def kernel(**inputs) -> np.ndarray:
    out, _ = run(inputs, trace=False)
    return out


# revision 11
# speedup vs baseline: 24883.9546x; 24883.9546x over previous
"""DiffOfGaussians Trainium2 kernel (v5: f16 stream, 2x-mode add trees).

Math:
  out[b,u] = sum_{h,w,c} inputs[b,h,w,c] * F[h,w,u] + bias[u]
  F[h,w,u] = g(a1,s1) - g(a2,s1+s2),  g(a,s) = a*exp(-((w-ux)^2+(h-uy)^2)/(2s))/(2*pi*s)

Separable filter: F[h,w,u] = Gx1[w,u]*gy1[u,h] + Gx2[w,u]*gy2[u,h] with the
amplitudes folded into gy and the minus sign into gy2.

Sharding: H split across 8 cores (16 rows each). The host pre-transposes each
slab to [h, w, b*c] float16: the DMA lands with w on partitions (half the HBM
bytes of f32) and the channel reduce is a stride-1 innermost halving tree of
tensor_tensor adds — measured to run in the DVE 2x f16 mode, unlike
tensor_reduce which stays at 1x on this hardware.

Per 4-row group: c-reduce tree (group 0 on GpSimd, rest on DVE) into
xt[w, h*64+b]; 4 f16 PE matmuls (2 paths x 2 unit-halves, paired per k into
one PSUM bank) produce pmm[u, (h,b)]; one wide DVE multiply per (k, group)
scales by gy[u,(p,h)] and scatters into slots[u, (b, hg,p,hh)]. A final
halving tree per k sums the 32 slot columns; bias/8 rides the f32 eviction
ACT. Host sums the 8 partial (2,128,64) outputs.
"""

import sys

for _p in ("/opt/trn_rl_repo",):
    if _p not in sys.path:
        sys.path.insert(0, _p)

import numpy as np

import concourse.bass as bass
import concourse.tile as tile
from concourse import bacc, masks, mybir
from concourse.bass_utils import run_bass_kernel_spmd

F32 = mybir.dt.float32
F16 = mybir.dt.float16
AX = mybir.AxisListType
OP = mybir.AluOpType
AF = mybir.ActivationFunctionType

B, H, W, C, U = 64, 128, 128, 16, 256
NCORES = 8
HSH = H // NCORES  # 16 rows per core
INV2PI = float(1.0 / (2.0 * np.pi))

_CACHE = {}


def _build_kernel():
    nc = bacc.Bacc(
        "TRN2",
        target_bir_lowering=False,
        debug=False,
        num_devices=NCORES,
    )

    # x transposed on host: x_d[h, w, b*c] (f16)
    x_d = nc.dram_tensor("x", [HSH, W, B * C], F16, kind="ExternalInput").ap()
    yc_d = nc.dram_tensor("yc", [1, HSH], F32, kind="ExternalInput").ap()
    # packed params: col 2i+k = param i, units k*128..k*128+127
    # order: a1, a2, s1, s2, ux, uy, bias (cols 12:14), pad to 16
    prm_d = nc.dram_tensor("prm", [128, 16], F32, kind="ExternalInput").ap()
    # out[k, u_lo, b] = partial of out[b, k*128+u_lo]
    out_d = nc.dram_tensor("out", [2, 128, 64], F32, kind="ExternalOutput").ap()

    GRP = 4  # h rows per group
    NG = HSH // GRP

    with tile.TileContext(nc) as tc:
        with (
            tc.tile_pool(name="singles", bufs=1) as singles,
            tc.tile_pool(name="gx", bufs=4) as gx_pool,
            tc.tile_pool(name="tree", bufs=2) as tree_pool,
            tc.tile_pool(name="ptr", bufs=2, space="PSUM") as tr_psum,
            tc.tile_pool(name="pmm", bufs=2, space="PSUM") as mm_psum,
        ):
            # input stream: 4 batched issues (4 h-rows each); the sync engine
            # takes ~660ns per DMA_DIRECT2D so 16 issues would stagger the
            # stream start over ~10us.
            xin = singles.tile([128, HSH * B * C], F16)
            hrow = W * B * C  # elements per h row in x_d
            for g in range(NG):
                src = bass.AP(
                    tensor=x_d.tensor,
                    offset=x_d.offset + g * GRP * hrow,
                    ap=[[B * C, W], [hrow, GRP], [1, B * C]],
                )
                nc.sync.dma_start(
                    out=xin[:, g * GRP * B * C : (g + 1) * GRP * B * C], in_=src
                )

            # ---------------- constants & parameters ----------------
            identity = singles.tile([128, 128], F32)
            masks.make_identity(nc, identity[:])

            zbias = singles.tile([128, 1], F32)
            nc.vector.memset(zbias[:], 0.0)

            iota_i = singles.tile([128, 128], mybir.dt.int32)
            nc.gpsimd.iota(iota_i[:], pattern=[[1, 128]], base=0, channel_multiplier=0)
            iota_f = singles.tile([128, 128], F32)
            nc.vector.tensor_copy(iota_f[:], iota_i[:])

            prm_sb = singles.tile([128, 16], F32)
            nc.scalar.dma_start(out=prm_sb[:], in_=prm_d)
            _ord = ("a1", "a2", "s1", "s2", "ux", "uy")
            psb = {n: prm_sb[:, 2 * i : 2 * i + 2] for i, n in enumerate(_ord)}
            bias_sb = prm_sb[:, 12:14]

            yc_sb = singles.tile([128, HSH], F32)
            yc_bcast = bass.AP(
                tensor=yc_d.tensor, offset=yc_d.offset, ap=[[0, 128], [1, HSH]]
            )
            nc.gpsimd.dma_start(out=yc_sb[:], in_=yc_bcast)

            # derived per-unit params, all [128, 2]
            sig2 = singles.tile([128, 2], F32)
            nc.vector.tensor_add(sig2[:], psb["s1"], psb["s2"])
            rc1 = singles.tile([128, 2], F32)
            nc.vector.reciprocal(rc1[:], psb["s1"])
            rc2 = singles.tile([128, 2], F32)
            nc.vector.reciprocal(rc2[:], sig2[:])
            nis = []  # -1/(2 sigma_path)
            for p, rc in enumerate((rc1, rc2)):
                t = singles.tile([128, 2], F32, tag=f"nis{p}")
                nc.vector.tensor_scalar_mul(t[:], rc[:], -0.5)
                nis.append(t)
            # amplitude coefs: c1 = a1/(2 pi s1), c2n = -a2/(2 pi (s1+s2))
            coef = []
            for p, (a, rc, s) in enumerate(
                ((psb["a1"], rc1, INV2PI), (psb["a2"], rc2, -INV2PI))
            ):
                t0 = singles.tile([128, 2], F32, tag=f"coefa{p}")
                nc.vector.tensor_mul(t0[:], a, rc[:])
                t1 = singles.tile([128, 2], F32, tag=f"coef{p}")
                nc.vector.tensor_scalar_mul(t1[:], t0[:], s)
                coef.append(t1)

            # ---------------- Gx tables: gxw[p][w, u] (f16) ----------------
            nux = singles.tile([128, 2], F32)
            nc.vector.tensor_scalar_mul(nux[:], psb["ux"], -1.0)
            nuy = singles.tile([128, 2], F32)
            nc.vector.tensor_scalar_mul(nuy[:], psb["uy"], -1.0)
            dx2 = []
            for k in range(2):
                d2 = singles.tile([128, 128], F32, tag=f"dx2_{k}")
                nc.scalar.activation(
                    d2[:], iota_f[:], AF.Square, bias=nux[:, k : k + 1]
                )
                dx2.append(d2)

            gxw = []  # per path: [128(w), 256(u)] f16
            for p in range(2):
                t = singles.tile([128, 256], F16, tag=f"gxw{p}")
                gxw.append(t)
            for p in range(2):
                for k in range(2):
                    g = gx_pool.tile([128, 128], F32, tag="gx")
                    nc.scalar.activation(
                        g[:], dx2[k][:], AF.Exp,
                        bias=zbias[:, 0:1], scale=nis[p][:, k : k + 1],
                    )
                    ps = tr_psum.tile([128, 128], F32)
                    nc.tensor.transpose(ps[:], g[:], identity[:])
                    nc.scalar.copy(gxw[p][:, k * 128 : (k + 1) * 128], ps[:])

            # ----- gy table: gy_all[u_lo, k*32 + p*16 + h] (f32) -----
            gy_all = singles.tile([128, 64], F32)
            for k in range(2):
                dy2 = gx_pool.tile([128, HSH], F32, tag="dy2")
                nc.scalar.activation(
                    dy2[:], yc_sb[:], AF.Square, bias=nuy[:, k : k + 1]
                )
                for p in range(2):
                    e = gx_pool.tile([128, HSH], F32, tag="gye")
                    nc.scalar.activation(
                        e[:], dy2[:], AF.Exp,
                        bias=zbias[:, 0:1], scale=nis[p][:, k : k + 1],
                    )
                    off = k * 32 + p * 16
                    nc.vector.tensor_scalar_mul(
                        gy_all[:, off : off + 16], e[:], coef[p][:, k : k + 1]
                    )

            bias8 = singles.tile([128, 2], F32)
            nc.vector.tensor_scalar_mul(bias8[:], bias_sb, 1.0 / NCORES)

            # xt[w, h*64+b] = sum_c x[h, w, b, c] (f16, matmul moving tensor)
            xt_all = singles.tile([128, HSH * 64], F16)
            # slots[k][u_lo, b*32 + hg*8 + p*4 + hh]
            slots = []
            for k in range(2):
                slot_k = singles.tile([128, 2048], F16, tag=f"slots{k}")
                slots.append(slot_k)

            def cred_tree(g, eng):
                """c-reduce rows [4g, 4g+4) via halving adds on `eng`."""
                n = GRP * B  # 256 (h,b) columns in this group
                src = xin[:, g * GRP * B * C : (g + 1) * GRP * B * C]
                cur = src.rearrange("q (n c) -> q n c", c=C)
                width = C
                while width > 2:
                    width //= 2
                    t = tree_pool.tile([128, n * width], F16, tag=f"tr{width}")
                    tv = t[:].rearrange("q (n c) -> q n c", c=width)
                    eng.tensor_add(tv, cur[:, :, 0:width], cur[:, :, width:])
                    cur = tv
                dst = xt_all[:, g * n : (g + 1) * n].rearrange(
                    "q (n c) -> q n c", c=1
                )
                eng.tensor_add(dst, cur[:, :, 0:1], cur[:, :, 1:2])

            def mm_group(hg):
                for k in range(2):
                    pmm = mm_psum.tile([128, 512], F32, tag="pmm")
                    for p in range(2):
                        nc.tensor.matmul(
                            pmm[:, p * 256 : (p + 1) * 256],
                            gxw[p][:, k * 128 : (k + 1) * 128],
                            xt_all[:, hg * 256 : (hg + 1) * 256],
                            start=True,
                            stop=True,
                        )
                    # wide multiplies: slots <- pmm * gy, dims (b, hh)
                    pq = pmm[:]
                    for p in range(2):
                        pv = bass.AP(
                            tensor=pq.tensor, offset=pq.offset + p * 256,
                            ap=[pq.ap[0], [1, 64], [64, 4]],
                        )
                        goff = k * 32 + p * 16 + hg * 4
                        gq = gy_all[:, goff : goff + 4]
                        gv = bass.AP(
                            tensor=gq.tensor, offset=gq.offset,
                            ap=[gq.ap[0], [0, 64], [1, 4]],
                        )
                        soff = hg * 8 + p * 4
                        sq = slots[k][:, soff : soff + 4]
                        sv = bass.AP(
                            tensor=sq.tensor, offset=sq.offset,
                            ap=[sq.ap[0], [32, 64], [1, 4]],
                        )
                        nc.vector.tensor_tensor(sv, pv, gv, op=OP.mult)

            with nc.allow_low_precision("f16 partials; harness gate is 2e-2"):
                for g in range(NG):
                    cred_tree(g, nc.gpsimd if g == 0 else nc.vector)
                    mm_group(g)

                # ---------------- final reduce + bias + store ----------------
                out_sb = singles.tile([128, 128], F32)
                for k in range(2):
                    cur = slots[k][:].rearrange("q (n r) -> q n r", r=32)
                    width = 32
                    while width > 2:
                        width //= 2
                        t = tree_pool.tile([128, 64 * width], F16, tag=f"fin{width}")
                        tv = t[:].rearrange("q (n r) -> q n r", r=width)
                        nc.vector.tensor_add(
                            tv, cur[:, :, 0:width], cur[:, :, width:]
                        )
                        cur = tv
                    sumk = singles.tile([128, 64], F16, tag=f"sum{k}")
                    nc.vector.tensor_add(
                        sumk[:].rearrange("q (n r) -> q n r", r=1),
                        cur[:, :, 0:1],
                        cur[:, :, 1:2],
                    )
                    nc.scalar.activation(
                        out_sb[:, k * 64 : (k + 1) * 64], sumk[:],
                        AF.Identity, bias=bias8[:, k : k + 1],
                    )
                    nc.sync.dma_start(
                        out=out_d[k], in_=out_sb[:, k * 64 : (k + 1) * 64]
                    )

    nc.compile()
    return nc


def _get_nc():
    if "nc" not in _CACHE:
        _CACHE["nc"] = _build_kernel()
    return _CACHE["nc"]


def pack_params(inputs: dict) -> np.ndarray:
    """[128, 16]: col 2i+k = param i (a1,a2,s1,s2,ux,uy,bias), unit block k."""
    prm = np.zeros((128, 16), dtype=np.float32)
    names = ("a1", "a2", "s1", "s2", "ux", "uy", "bias")
    for i, n in enumerate(names):
        v = np.asarray(inputs[n], dtype=np.float32).reshape(U)
        prm[:, 2 * i] = v[:128]
        prm[:, 2 * i + 1] = v[128:]
    return prm


def run(inputs: dict, trace: bool = False):
    """Run on 8 cores; returns (full_output, BassKernelResults)."""
    nc = _get_nc()
    x = np.asarray(inputs["inputs"], dtype=np.float32)
    # [b, h, w, c] -> [h, w, b, c], cast f16; per-core slabs are contiguous
    xt = np.ascontiguousarray(
        x.transpose(1, 2, 0, 3).astype(np.float16)
    ).reshape(H, W, B * C)
    prm = pack_params(inputs)
    in_maps = []
    for i in range(NCORES):
        m = {
            "x": xt[i * HSH : (i + 1) * HSH],
            "yc": np.arange(i * HSH, (i + 1) * HSH, dtype=np.float32).reshape(
                1, HSH
            ),
            "prm": prm,
        }
        in_maps.append(m)

    res = run_bass_kernel_spmd(
        nc, in_maps, core_ids=list(range(NCORES)), trace=trace
    )
    # partials: [2, 128, 64] -> out[b, k*128+u_lo]
    total = np.zeros((2, 128, 64), dtype=np.float64)
    for r in res.results:
        total += r["out"].astype(np.float64)
    out = total.transpose(2, 0, 1).reshape(64, 256).astype(np.float32)
    return out, res


def kernel(**inputs) -> np.ndarray:
    out, _ = run(inputs, trace=False)
    return out
